# revision 8
# baseline (speedup 1.0000x reference)
"""TRN2 Bass kernel for nn_CosFreqEncoding: out = ((x @ W.T) @ cos_basis) / max.

Strategy: data-parallel over batch across 8 NeuronCores. Each core computes
its 512-row shard of both GEMMs in bf16 (fp32 PSUM accumulation, which keeps
rel err ~4e-3, well under the 2e-2 gate), a local max, one scalar
AllReduce(max), then scales and writes its output shard in bf16 (host
converts to fp32 after the gather — the normalization makes any global
positive scaling wash out, and bf16 quantization of values in [-1,1] adds
<2^-9 abs error).

bf16 vs the old fp32r build: half the HBM traffic (24MB -> fully hidden
under compute) and TensorE fast-weight-load (FWL) is enabled, so LDWEIGHTS
(~217ns at fp32r, longer than the 213ns MM-512) stops stalling the PE.

Layouts (host-prepped so no on-chip transposes are needed):
  GEMM1: xfT[f, m] += W.T[l, f].T @ x.T[l, m]   (lhsT = W.T block, rhs = x.T)
  GEMM2: out[m, l2] += xfT[f, m].T @ cos[f, l2] (lhsT = xfT slice, rhs = cos)

The global max is exchanged with ONE AllReduce (the old build used two,
serialized on the single CC stream at ~30us each). gpsimd carries no DMA
work so its collective trigger fires immediately after the last local max.

Self-contained: hardcodes shapes from the problem spec.
"""
import numpy as np
import ml_dtypes

import concourse.bass as bass
import concourse.bass_isa as bass_isa
import concourse.bacc as bacc
import concourse.mybir as mybir
import concourse.tile as tile
import concourse.bass_utils as bass_utils

N_CORES = 8
B, L, F = 4096, 2048, 2074
FP = 2176               # F padded to 17 full 128-tiles
BS = B // N_CORES       # 512 batch rows per core
LT = L // 128           # 16 l-tiles (GEMM1 contraction)
FT = FP // 128          # 17 f-tiles
MT = BS // 128          # 4 m-tiles
CK = L // 512           # 4 output column chunks of 512
F32 = mybir.dt.float32
BF16 = mybir.dt.bfloat16
NP_BF16 = ml_dtypes.bfloat16


def _emit(nc, tc, xT, Wb, cosb, out):
    with (
        tc.tile_pool(name="xp", bufs=1) as xp,
        tc.tile_pool(name="wp", bufs=3) as wp,
        tc.tile_pool(name="xfp", bufs=1) as xfp,
        tc.tile_pool(name="cp", bufs=2) as cp,
        tc.tile_pool(name="op", bufs=1) as op,
        tc.tile_pool(name="sp", bufs=1) as sp,
        tc.tile_pool(name="ps1", bufs=2, space="PSUM") as ps1,
        tc.tile_pool(name="ps2", bufs=6, space="PSUM") as ps2,
        tc.tile_pool(name="dp", bufs=1, space="DRAM") as dp,
    ):
        # DMA issue queues: sync + scalar carry the bulk input streams.
        # gpsimd stays empty so the AllReduce trigger is never queued behind
        # descriptor generation.

        # resident x.T: one [128, 16*512] tile, filled by 4 quarter-DMAs so
        # GEMM1's first matmuls start after ~1.5us instead of ~6us.
        xt = xp.tile([128, LT * BS], BF16, name="xt")
        xtv = xt[:].rearrange("p (li b) -> p li b", li=LT)
        for qi in range(4):
            (nc.sync if qi % 2 == 0 else nc.scalar).dma_start(
                xtv[:, qi * 4:(qi + 1) * 4], xT[:, qi * 4:(qi + 1) * 4])

        # cos chunk 0 prefetch (2.2MB) on gpsimd's queue, early and hidden.
        # gpsimd's last cos issue lands ~40us before the AllReduce trigger,
        # so the trigger is never queued behind descriptor generation.
        ct = [cp.tile([128, FT * 512], BF16, tag="cos", name=f"ct{ci % 2}")
              for ci in range(2)]
        nc.gpsimd.dma_start(ct[0][:], cosb[0])

        # GEMM1: xfT[f-tile] [128 f, 512 m]; W streamed one 512KB DMA per
        # f-tile column (16 lhsT blocks each).
        xf = [xfp.tile([128, BS], BF16, name=f"xf{fi}") for fi in range(FT)]
        for fi in range(FT):
            ps = ps1.tile([128, BS], F32, tag="g1")
            wcol = wp.tile([128, LT * 128], BF16, tag="w")
            (nc.scalar if fi % 2 == 0 else nc.sync).dma_start(
                wcol[:].rearrange("p (li b) -> p li b", li=LT), Wb[fi])
            for li in range(LT):
                nc.tensor.matmul(ps[:], wcol[:, li * 128:(li + 1) * 128],
                                 xt[:, li * BS:(li + 1) * BS],
                                 start=(li == 0), stop=(li == LT - 1))
            # cast fp32 -> bf16 while copying out of PSUM
            nc.vector.tensor_copy(xf[fi][:], ps[:])

        # GEMM2 + fused local max. Chunk ci's cos block arrives as one
        # 2.2MB DMA prefetched during chunk ci-1 (or GEMM1 for ci<=1).
        nc.gpsimd.dma_start(ct[1][:], cosb[1])
        ot = [op.tile([128, L], BF16, name=f"ot{mi}") for mi in range(MT)]
        vmaxes = sp.tile([128, MT * CK], F32)
        for ci in range(CK):
            c = ct[ci % 2]
            # mi outer: each PSUM bank takes 17 back-to-back accumulating
            # matmuls (no per-instruction bank cycling, which triggers HAM
            # re-throttling), and bank mi drains while mi+1 still matmuls.
            for mi in range(MT):
                pst = ps2.tile([128, 512], F32, tag="g2",
                               name=f"ps2_{ci}_{mi}")
                for fi in range(FT):
                    nc.tensor.matmul(
                        pst[:], xf[fi][:, mi * 128:(mi + 1) * 128],
                        c[:, fi * 512:(fi + 1) * 512],
                        start=(fi == 0), stop=(fi == FT - 1))
                # prefetch emitted only after the chunk's LAST matmul: a
                # tile-write is ordered before later-emitted readers, so an
                # earlier emission would feed chunk ci+2's cos to mi>0.
                if mi == MT - 1 and ci + 2 < CK:
                    nc.gpsimd.dma_start(ct[ci % 2][:], cosb[ci + 2])
                idx = ci * MT + mi
                nc.vector.reduce_max(vmaxes[:, idx:idx + 1], pst[:],
                                     axis=mybir.AxisListType.X)
                nc.vector.tensor_copy(ot[mi][:, ci * 512:(ci + 1) * 512],
                                      pst[:])

        # single scalar AllReduce(max); gpsimd queue is nearly empty so the
        # trigger fires right after the final local reduce. The local scalar
        # max is vector XYZW + partition_all_reduce (gpsimd XYZWC is slow).
        lm = sp.tile([128, 1], F32)
        nc.vector.reduce_max(lm[:], vmaxes[:], axis=mybir.AxisListType.X)
        lmb = sp.tile([128, 1], F32)
        nc.gpsimd.partition_all_reduce(lmb[:], lm[:], channels=128,
                                       reduce_op=bass_isa.ReduceOp.max)
        cc_in = dp.tile([1], F32, name="ccin")
        cc_out = dp.tile([1], F32, name="ccout")
        nc.gpsimd.dma_start(cc_in[:], lmb[0:1, 0])
        nc.gpsimd.collective_compute(
            "AllReduce", mybir.AluOpType.max,
            replica_groups=[list(range(N_CORES))],
            ins=[cc_in[:]], outs=[cc_out[:]])
        gbc = sp.tile([128, 1], F32)
        nc.gpsimd.dma_start(gbc[:], cc_out[:].partition_broadcast(128))
        rbc = sp.tile([128, 1], F32)
        nc.vector.reciprocal(rbc[:], gbc[:])

        # scale + store; full 4KB-per-partition rows so each write is one
        # 512KB descriptor-friendly DMA.
        for mi in range(MT):
            nc.vector.tensor_scalar_mul(ot[mi][:], ot[mi][:], rbc[:, 0:1])
            (nc.sync if mi % 2 == 0 else nc.scalar).dma_start(
                out[mi * 128:(mi + 1) * 128, :], ot[mi][:])


def _build():
    nc = bacc.Bacc("TRN2", target_bir_lowering=False, debug=False,
                   num_devices=N_CORES)
    xT = nc.dram_tensor("xT", [128, LT, BS], BF16, kind="ExternalInput")
    Wb = nc.dram_tensor("Wb", [FT, 128, LT, 128], BF16, kind="ExternalInput")
    cosb = nc.dram_tensor("cosb", [CK, 128, FT, 512], BF16,
                          kind="ExternalInput")
    out = nc.dram_tensor("out", [BS, L], BF16, kind="ExternalOutput")
    with tile.TileContext(nc) as tc:
        _emit(nc, tc, xT, Wb, cosb, out)
    nc.compile()
    return nc


_cached_nc = None


def _get_nc():
    global _cached_nc
    if _cached_nc is None:
        _cached_nc = _build()
    return _cached_nc


def _prep_inputs(x, W, cos_basis):
    x = np.ascontiguousarray(x, dtype=np.float32)
    W = np.ascontiguousarray(W, dtype=np.float32)
    cos = np.ascontiguousarray(cos_basis, dtype=np.float32)
    # pad freq dim to FP with zeros
    Wp = np.zeros((FP, L), dtype=np.float32)
    Wp[:F] = W
    cosp = np.zeros((FP, L), dtype=np.float32)
    cosp[:F] = cos
    # Wb[fi, p, li, b] = W.T[li*128+p, fi*128+b] = Wp[fi*128+b, li*128+p]
    Wb = np.ascontiguousarray(
        Wp.reshape(FT, 128, LT, 128).transpose(0, 3, 2, 1).astype(NP_BF16))
    # cosb[ci, p, fi, n] = cosp[fi*128+p, ci*512+n]
    cosb = np.ascontiguousarray(
        cosp.reshape(FT, 128, CK, 512).transpose(2, 1, 0, 3).astype(NP_BF16))
    # xT[p, li, m] = x_shard[m, li*128+p]
    xTs = []
    for i in range(N_CORES):
        sh = x[i * BS:(i + 1) * BS].reshape(BS, LT, 128)
        xTs.append(np.ascontiguousarray(
            sh.transpose(2, 1, 0).astype(NP_BF16)))
    return xTs, Wb, cosb


def kernel(x, W, cos_basis, _trace=False, _trace_kwargs=None):
    xTs, Wb, cosb = _prep_inputs(x, W, cos_basis)
    nc = _get_nc()
    in_maps = [{"xT": xTs[i], "Wb": Wb, "cosb": cosb} for i in range(N_CORES)]
    res = bass_utils.run_bass_kernel_spmd(
        nc, in_maps, core_ids=list(range(N_CORES)), trace=_trace,
        **(_trace_kwargs or {}))
    out = np.concatenate(
        [res.results[i]["out"].astype(np.float32) for i in range(N_CORES)],
        axis=0)
    if _trace:
        kernel.last_result = res
    return out
